# revision 5
# baseline (speedup 1.0000x reference)
"""Trainium2 Bass kernel for the batched linear-chain CRF NLL.

Math: the reference computes, per batch row b,
    NLL[b] = logZ[b] - gold[b]
where logZ is the CRF forward (log-partition) over S=1024 steps with T=73
tags, and gold is the score of the labeled path.

Device strategy (pure data parallelism, batch 256 -> 32 rows x 8 cores):
  * Forward recurrence runs in exp-space:  A_{t+1} = E_t (.) (Mt^T A_t)
    with A in transposed layout [73 tags (partitions) x 32 batch (free)],
    Mt = exp(trans - kappa) stationary-ish on the TensorEngine, E_t =
    exp(feats_t)^T produced by PE transposes (matmul with identity) + one
    bulk ScalarEngine exp per 16 steps.  One VectorEngine tensor_tensor
    multiply per step carries the serial chain.
  * Numerical scaling: every RS steps a column-sum (ones matmul) +
    reciprocal + rank-1 broadcast matmul prescales a *future* E slice, so
    the rescale never adds latency to the serial chain.  log(c) factors
    are accumulated at the end from the stored reciprocals.
  * Gold score: two indirect-DMA gathers (feats at gold tags; a combined
    transition/start/stop table at host-computed flat indices), free-dim
    reduces, and a selection matmul to fold partial sums per row.

The kernel is self-contained: shapes/sharding hardcoded for
feats[256,1024,73], mask all-ones, tags[256,1024].
"""
import numpy as np

import concourse.mybir as mybir
import concourse.tile as tile
from concourse import bacc
import concourse.bass as bass
from concourse.bass_utils import run_bass_kernel_spmd

F32 = mybir.dt.float32
I32 = mybir.dt.int32

B, S, T = 256, 1024, 73
NCORES = 8
BC = B // NCORES          # 32 batch rows per core
CH = 64                   # feats steps per DMA chunk
TCH = 16                  # steps per PSUM transpose bank / ACT exp
RS = 32                   # rescale period (steps)


def _build_nc(s_len: int):
    """Build the per-core Bass program for a sequence of s_len steps."""
    n_chunks = s_len // CH
    n_resc = (s_len - 3) // RS + 1 if s_len > 32 else 1
    # rescale at steps t = RS-3, 2*RS-3, ... (t+2 stays inside t's chunk)
    resc_steps = set(range(RS - 3, s_len - 2, RS))
    nr = len(resc_steps)
    gw = 257 if s_len == S else (s_len + 1 + 2 + 3) // 4  # gather width/row
    # pad so 4*gw >= s_len+2 entries (trans pairs + start + stop)
    while 4 * gw < (s_len - 1) + 2:
        gw += 1

    nc = bacc.Bacc(None, target_bir_lowering=False)
    with tile.TileContext(nc) as tc:
        with tc.tile_pool(name="dram", bufs=1, space="DRAM") as dram:
            fflat = dram.tile([BC * s_len * T], F32, kind="ExternalInput",
                              name="fflat", uniquify=False)
            mt = dram.tile([T, T], F32, kind="ExternalInput", name="mt",
                           uniquify=False)
            ident = dram.tile([32, 32], F32, kind="ExternalInput", name="ident",
                              uniquify=False)
            est = dram.tile([T, 1], F32, kind="ExternalInput", name="est",
                            uniquify=False)
            esp = dram.tile([T, 1], F32, kind="ExternalInput", name="esp",
                            uniquify=False)
            onescol = dram.tile([T, 1], F32, kind="ExternalInput", name="onescol",
                                uniquify=False)
            onesrow = dram.tile([1, T], F32, kind="ExternalInput", name="onesrow",
                                uniquify=False)
            iota = dram.tile([BC, T], F32, kind="ExternalInput", name="iota",
                             uniquify=False)
            tagsf = dram.tile([BC, s_len], F32, kind="ExternalInput",
                              name="tagsf", uniquify=False)
            out = dram.tile([1, BC], F32, kind="ExternalOutput", name="out",
                            uniquify=False)

        with (
            tc.tile_pool(name="const", bufs=1) as cp,
            tc.tile_pool(name="fnat", bufs=2) as fp,
            tc.tile_pool(name="et", bufs=2) as ep,
            tc.tile_pool(name="state", bufs=3) as stp,
            tc.tile_pool(name="psl", bufs=2) as pslp,
            tc.tile_pool(name="misc", bufs=1) as mp,
            tc.tile_pool(name="ps_s", bufs=2, space="PSUM") as pss,
            tc.tile_pool(name="ps_f", bufs=2, space="PSUM") as psf,
            tc.tile_pool(name="ps_r", bufs=1, space="PSUM") as psr,
            tc.tile_pool(name="ps_g", bufs=1, space="PSUM") as psg,
        ):
            # ---- constants into SBUF ----
            mt_s = cp.tile([T, T], F32)
            nc.sync.dma_start(mt_s[:], mt[:])
            id_s = cp.tile([32, 32], F32)
            nc.sync.dma_start(id_s[:], ident[:])
            est_s = cp.tile([T, 1], F32)
            nc.sync.dma_start(est_s[:], est[:])
            esp_s = cp.tile([T, 1], F32)
            nc.sync.dma_start(esp_s[:], esp[:])
            oc_s = cp.tile([T, 1], F32)
            nc.sync.dma_start(oc_s[:], onescol[:])
            or_s = cp.tile([1, T], F32)
            nc.sync.dma_start(or_s[:], onesrow[:])
            iota_s = cp.tile([BC, T], F32)
            nc.sync.dma_start(iota_s[:], iota[:])
            tags_s = cp.tile([BC, s_len], F32)
            nc.sync.dma_start(tags_s[:], tagsf[:])
            ring = mp.tile([1, 32 * max(nr, 1)], F32)
            partials = mp.tile([BC, s_len], F32)
            scratch = mp.tile([BC, T], F32)


            # ---- forward recurrence ----
            state = None
            ridx = 0
            pending = {}  # step -> prescaled E tile
            for c in range(n_chunks):
                fn = fp.tile([BC, CH * T], F32, tag="fnat")
                # feats[b, c*CH:(c+1)*CH, :] contiguous per row
                nc.sync.dma_start(
                    fn[:], fflat[:].rearrange("(b r) -> b r", b=BC)
                    [:, c * CH * T:(c + 1) * CH * T])
                et = ep.tile([T, CH * 32], F32, tag="et")
                for g in range(CH // TCH):
                    ftp = psf.tile([T, TCH * 32], F32, tag="ftp")
                    for k in range(TCH):
                        kk = g * TCH + k
                        nc.tensor.matmul(
                            ftp[:, k * 32:(k + 1) * 32],
                            lhsT=fn[:, kk * T:(kk + 1) * T], rhs=id_s[:],
                            start=True, stop=True)
                    nc.scalar.activation(
                        et[:, g * TCH * 32:(g + 1) * TCH * 32], ftp[:],
                        mybir.ActivationFunctionType.Exp)
                for k in range(CH):
                    t = c * CH + k
                    # gold feat-score contribution for step t (off-chain)
                    nc.vector.scalar_tensor_tensor(
                        out=scratch[:], in0=iota_s[:],
                        scalar=tags_s[:, t:t + 1],
                        in1=fn[:, k * T:(k + 1) * T],
                        op0=mybir.AluOpType.is_equal,
                        op1=mybir.AluOpType.mult,
                        accum_out=partials[:, t:t + 1])
                    esl = pending.pop(t, None)
                    esl = esl if esl is not None else et[:, k * 32:(k + 1) * 32]
                    if t == 0:
                        state = stp.tile([T, 32], F32, tag="state")
                        nc.vector.tensor_scalar(
                            out=state[:], in0=esl, scalar1=est_s[:, 0:1],
                            scalar2=None, op0=mybir.AluOpType.mult)
                        continue
                    sp = pss.tile([T, 32], F32, tag="sp")
                    nc.tensor.matmul(sp[:], lhsT=mt_s[:], rhs=state[:],
                                     start=True, stop=True)
                    nstate = stp.tile([T, 32], F32, tag="state")
                    nc.vector.tensor_tensor(out=nstate[:], in0=sp[:], in1=esl,
                                            op=mybir.AluOpType.mult)
                    state = nstate
                    if t in resc_steps:
                        # rescale side-chain: prescale E slice for step t+2
                        cps = psr.tile([1, 32], F32, tag="cps")
                        nc.tensor.matmul(cps[:], lhsT=oc_s[:], rhs=state[:],
                                         start=True, stop=True)
                        rsl = ring[:, ridx * 32:(ridx + 1) * 32]
                        nc.vector.reciprocal(rsl, cps[:])
                        cb = psr.tile([T, 32], F32, tag="cb")
                        nc.tensor.matmul(cb[:], lhsT=or_s[:], rhs=rsl,
                                         start=True, stop=True)
                        k2 = k + 2
                        psl = pslp.tile([T, 32], F32, tag="psl")
                        nc.vector.tensor_tensor(
                            out=psl[:], in0=et[:, k2 * 32:(k2 + 1) * 32],
                            in1=cb[:], op=mybir.AluOpType.mult)
                        pending[t + 2] = psl[:]
                        ridx += 1

            # ---- finale: gold feat-score fold + transpose ----
            gold_sb = mp.tile([BC, 1], F32)
            nc.vector.tensor_reduce(gold_sb[:], partials[:],
                                    axis=mybir.AxisListType.X,
                                    op=mybir.AluOpType.add)
            goldT_ps = psg.tile([1, 32], F32)
            nc.tensor.matmul(goldT_ps[:], lhsT=gold_sb[:], rhs=id_s[:],
                             start=True, stop=True)
            goldT = mp.tile([1, 32], F32)
            nc.vector.tensor_copy(goldT[:], goldT_ps[:])
            sdot = psr.tile([1, 32], F32, tag="cps")
            nc.tensor.matmul(sdot[:], lhsT=esp_s[:], rhs=state[:],
                             start=True, stop=True)
            lnf = mp.tile([1, 32], F32)
            nc.scalar.activation(lnf[:], sdot[:],
                                 mybir.ActivationFunctionType.Ln)
            if ridx > 0:
                lnring = mp.tile([1, 32 * nr], F32)
                nc.scalar.activation(lnring[:], ring[:, :32 * nr],
                                     mybir.ActivationFunctionType.Ln)
                lnsum = mp.tile([1, 32], F32)
                nc.vector.tensor_reduce(
                    lnsum[:],
                    lnring[:].rearrange("p (r b) -> p b r", b=32),
                    axis=mybir.AxisListType.X, op=mybir.AluOpType.add)
                y0 = mp.tile([1, 32], F32)
                # lnsum holds sum of ln(1/c): logZ = lnf - lnsum + (s-1)*kappa
                nc.vector.tensor_tensor(out=y0[:], in0=lnf[:], in1=lnsum[:],
                                        op=mybir.AluOpType.subtract)
            else:
                y0 = lnf
            y1 = mp.tile([1, 32], F32)
            nc.vector.tensor_tensor(out=y1[:], in0=y0[:], in1=goldT[:],
                                    op=mybir.AluOpType.subtract)
            nc.sync.dma_start(out[:], y1[:])
    nc.compile()
    return nc


_NC_CACHE = {}


def _get_nc(s_len):
    if s_len not in _NC_CACHE:
        _NC_CACHE[s_len] = _build_nc(s_len)
    return _NC_CACHE[s_len]


def _host_constants(cdt, types0, types1, start_t, stop_t):
    trans = np.asarray(cdt, np.float64)[np.asarray(types0), np.asarray(types1)]
    kappa = float(np.log(np.exp(trans).sum(1)).mean() + 0.5)
    mt_np = np.exp(trans - kappa).astype(np.float32)          # [T,T] lhsT=[i,j]
    est_np = np.exp(np.asarray(start_t, np.float32)).reshape(T, 1)
    esp_np = np.exp(np.asarray(stop_t, np.float32)).reshape(T, 1)
    return mt_np, est_np, esp_np, kappa, trans


def kernel(feats, mask, tags, cdt_transitions, start_transitions,
           stop_transitions, types0, types1, s_len=None):
    feats = np.asarray(feats, np.float32)
    tags = np.asarray(tags, np.int64)
    s_len = feats.shape[1] if s_len is None else s_len
    mt_np, est_np, esp_np, kappa, trans = _host_constants(
        cdt_transitions, types0, types1, start_transitions, stop_transitions)
    start64 = np.asarray(start_transitions, np.float64)
    stop64 = np.asarray(stop_transitions, np.float64)
    # tags-only part of the gold score (host; touches no large tensors)
    gs = (trans[tags[:, :s_len - 1], tags[:, 1:s_len]].sum(1)
          + start64[tags[:, 0]] + stop64[tags[:, s_len - 1]])
    nc = _get_nc(s_len)
    iota_np = np.broadcast_to(np.arange(T, dtype=np.float32), (BC, T)).copy()
    in_maps = []
    for c in range(NCORES):
        in_maps.append({
            "fflat": np.ascontiguousarray(
                feats[c * BC:(c + 1) * BC, :s_len]).reshape(-1),
            "mt": mt_np, "ident": np.eye(32, dtype=np.float32),
            "est": est_np, "esp": esp_np,
            "onescol": np.ones((T, 1), np.float32),
            "onesrow": np.ones((1, T), np.float32),
            "iota": iota_np,
            "tagsf": tags[c * BC:(c + 1) * BC, :s_len].astype(np.float32),
        })
    res = run_bass_kernel_spmd(nc, in_maps, core_ids=list(range(NCORES)))
    outs = [res.results[c]["out"].reshape(BC) for c in range(NCORES)]
    nll = np.concatenate(outs).astype(np.float64)
    # logZ kappa correction and tags-only gold part
    nll = nll + (s_len - 1) * kappa - gs
    return nll.astype(np.float32)


# revision 8
# speedup vs baseline: 290.3327x; 290.3327x over previous
"""Trainium2 Bass kernel for the batched linear-chain CRF NLL.

Math: the reference computes, per batch row b,
    NLL[b] = logZ[b] - gold[b]
where logZ is the CRF forward (log-partition) over S=1024 steps with T=73
tags, and gold is the score of the labeled path.

Device strategy (pure data parallelism, batch 256 -> 32 rows x 8 cores):
  * Forward recurrence runs in exp-space:  A_{t+1} = E_t (.) (Mt^T A_t)
    with A in transposed layout [73 tags (partitions) x 32 batch (free)],
    Mt = exp(trans - kappa) stationary-ish on the TensorEngine, E_t =
    exp(feats_t)^T produced by PE transposes (matmul with identity) + one
    bulk ScalarEngine exp per 16 steps.  One VectorEngine tensor_tensor
    multiply per step carries the serial chain.
  * Numerical scaling: every RS steps a column-sum (ones matmul) +
    reciprocal + rank-1 broadcast matmul prescales a *future* E slice, so
    the rescale never adds latency to the serial chain.  log(c) factors
    are accumulated at the end from the stored reciprocals.
  * Gold score: two indirect-DMA gathers (feats at gold tags; a combined
    transition/start/stop table at host-computed flat indices), free-dim
    reduces, and a selection matmul to fold partial sums per row.

The kernel is self-contained: shapes/sharding hardcoded for
feats[256,1024,73], mask all-ones, tags[256,1024].
"""
import numpy as np

import concourse.mybir as mybir
import concourse.tile as tile
from concourse import bacc
import concourse.bass as bass
from concourse.bass_utils import run_bass_kernel_spmd

F32 = mybir.dt.float32
I32 = mybir.dt.int32

B, S, T = 256, 1024, 73
NCORES = 8
BC = B // NCORES          # 32 batch rows per core
CH = 64                   # feats steps per DMA chunk
TCH = 16                  # steps per PSUM transpose bank / ACT exp
RS = 32                   # rescale period (steps)


def _build_nc(s_len: int, reps: int = 1):
    """Build the per-core Bass program for a sequence of s_len steps."""
    n_chunks = s_len // CH
    n_resc = (s_len - 3) // RS + 1 if s_len > 32 else 1
    # rescale at steps t = RS-3, 2*RS-3, ... (t+2 stays inside t's chunk)
    resc_steps = set(range(RS - 3, s_len - 2, RS))
    nr = len(resc_steps)
    gw = 257 if s_len == S else (s_len + 1 + 2 + 3) // 4  # gather width/row
    # pad so 4*gw >= s_len+2 entries (trans pairs + start + stop)
    while 4 * gw < (s_len - 1) + 2:
        gw += 1

    nc = bacc.Bacc(None, target_bir_lowering=False)
    with tile.TileContext(nc) as tc:
        with tc.tile_pool(name="dram", bufs=1, space="DRAM") as dram:
            fflat = dram.tile([BC * s_len * T], F32, kind="ExternalInput",
                              name="fflat", uniquify=False)
            mt = dram.tile([T, T], F32, kind="ExternalInput", name="mt",
                           uniquify=False)
            ident = dram.tile([32, 32], F32, kind="ExternalInput", name="ident",
                              uniquify=False)
            est = dram.tile([T, 1], F32, kind="ExternalInput", name="est",
                            uniquify=False)
            esp = dram.tile([T, 1], F32, kind="ExternalInput", name="esp",
                            uniquify=False)
            onescol = dram.tile([T, 1], F32, kind="ExternalInput", name="onescol",
                                uniquify=False)
            onesrow = dram.tile([1, T], F32, kind="ExternalInput", name="onesrow",
                                uniquify=False)
            iota = dram.tile([BC, T], F32, kind="ExternalInput", name="iota",
                             uniquify=False)
            tagsf = dram.tile([BC, s_len], F32, kind="ExternalInput",
                              name="tagsf", uniquify=False)
            out = dram.tile([1, BC], F32, kind="ExternalOutput", name="out",
                            uniquify=False)

        with (
            tc.tile_pool(name="const", bufs=1) as cp,
            tc.tile_pool(name="fnat", bufs=2) as fp,
            tc.tile_pool(name="et", bufs=3) as ep,
            tc.tile_pool(name="state", bufs=3) as stp,
            tc.tile_pool(name="psl", bufs=2) as pslp,
            tc.tile_pool(name="misc", bufs=1) as mp,
            tc.tile_pool(name="ps_s", bufs=2, space="PSUM") as pss,
            tc.tile_pool(name="ps_f", bufs=2, space="PSUM") as psf,
            tc.tile_pool(name="ps_r", bufs=1, space="PSUM") as psr,
            tc.tile_pool(name="ps_g", bufs=1, space="PSUM") as psg,
        ):
            # ---- constants into SBUF ----
            mt_s = cp.tile([T, T], F32)
            nc.sync.dma_start(mt_s[:], mt[:])
            id128 = cp.tile([128, 32], F32)
            id_s = id128[96:128, :]
            nc.sync.dma_start(id_s, ident[:])
            est_s = cp.tile([T, 1], F32)
            nc.sync.dma_start(est_s[:], est[:])
            esp_s = cp.tile([T, 1], F32)
            nc.sync.dma_start(esp_s[:], esp[:])
            oc_s = cp.tile([T, 1], F32)
            nc.sync.dma_start(oc_s[:], onescol[:])
            or_s = cp.tile([1, T], F32)
            nc.sync.dma_start(or_s[:], onesrow[:])
            iota128 = cp.tile([128, T], F32)
            iota_s = iota128[96:128, :]
            nc.sync.dma_start(iota_s, iota[:])
            tags128 = cp.tile([128, s_len], F32)
            tags_s = tags128[96:128, :]
            nc.sync.dma_start(tags_s, tagsf[:])
            ring = mp.tile([1, 32 * max(nr, 1)], F32)
            partials128 = mp.tile([128, s_len], F32)
            partials = partials128[96:128, :]
            scratch128 = mp.tile([128, T], F32)
            scratch = scratch128[96:128, :]


            # ---- forward recurrence ----
            import contextlib
            rep_cm = (tc.For_i(0, reps, 1) if reps > 1
                      else contextlib.nullcontext())
            state = None
            ridx = 0
            pending = {}  # step -> prescaled E tile
            rep_cm.__enter__()
            for c in range(n_chunks):
                fn128 = fp.tile([128, CH * T], F32, tag="fnat")
                fn = fn128[96:128, :]
                # feats[b, c*CH:(c+1)*CH, :] contiguous per row
                nc.sync.dma_start(
                    fn, fflat[:].rearrange("(b r) -> b r", b=BC)
                    [:, c * CH * T:(c + 1) * CH * T])
                et = ep.tile([T, CH * 32], F32, tag="et")
                for g in range(CH // TCH):
                    ftp = psf.tile([T, TCH * 32], F32, tag="ftp")
                    for k in range(TCH):
                        kk = g * TCH + k
                        nc.tensor.matmul(
                            ftp[:, k * 32:(k + 1) * 32],
                            lhsT=fn[:, kk * T:(kk + 1) * T], rhs=id_s,
                            start=True, stop=True, tile_position=(96, 0))
                    nc.scalar.activation(
                        et[:, g * TCH * 32:(g + 1) * TCH * 32], ftp[:],
                        mybir.ActivationFunctionType.Exp)
                for k in range(CH):
                    t = c * CH + k
                    esl = pending.pop(t, None)
                    esl = esl if esl is not None else et[:, k * 32:(k + 1) * 32]
                    if t == 0:
                        state = stp.tile([T, 32], F32, tag="state")
                        nc.vector.tensor_scalar(
                            out=state[:], in0=esl, scalar1=est_s[:, 0:1],
                            scalar2=None, op0=mybir.AluOpType.mult)
                    else:
                        sp = pss.tile([T, 32], F32, tag="sp")
                        nc.tensor.matmul(sp[:], lhsT=mt_s[:], rhs=state[:],
                                         start=True, stop=True)
                        nstate = stp.tile([T, 32], F32, tag="state")
                        nc.vector.tensor_tensor(out=nstate[:], in0=sp[:],
                                                in1=esl,
                                                op=mybir.AluOpType.mult)
                        state = nstate
                    # gold feat-score contribution for step t (off-chain)
                    nc.vector.scalar_tensor_tensor(
                        out=scratch, in0=iota_s,
                        scalar=tags_s[:, t:t + 1],
                        in1=fn[:, k * T:(k + 1) * T],
                        op0=mybir.AluOpType.is_equal,
                        op1=mybir.AluOpType.mult,
                        accum_out=partials[:, t:t + 1])
                    if t == 0:
                        continue
                    if t in resc_steps:
                        # rescale side-chain: prescale E slice for step t+2
                        cc = psr.tile([T, 32], F32, tag="cc")
                        nc.tensor.matmul(cc[0:1, :], lhsT=oc_s[:], rhs=state[:],
                                         start=True, stop=True)
                        rsl = ring[:, ridx * 32:(ridx + 1) * 32]
                        nc.vector.reciprocal(rsl, cc[0:1, :])
                        cb = psr.tile([T, 32], F32, tag="cc")
                        nc.tensor.matmul(cb[:], lhsT=or_s[:], rhs=rsl,
                                         start=True, stop=True)
                        k2 = k + 2
                        psl = pslp.tile([T, 32], F32, tag="psl")
                        nc.vector.tensor_tensor(
                            out=psl[:], in0=et[:, k2 * 32:(k2 + 1) * 32],
                            in1=cb[:], op=mybir.AluOpType.mult)
                        pending[t + 2] = psl[:]
                        ridx += 1

            # ---- finale: gold feat-score fold + transpose ----
            gold128 = mp.tile([128, 1], F32)
            gold_sb = gold128[96:128, :]
            nc.vector.tensor_reduce(gold_sb, partials,
                                    axis=mybir.AxisListType.X,
                                    op=mybir.AluOpType.add)
            goldT_ps = psg.tile([1, 32], F32)
            nc.tensor.matmul(goldT_ps[:], lhsT=gold_sb, rhs=id_s,
                             start=True, stop=True, tile_position=(96, 0))
            goldT = mp.tile([1, 32], F32)
            nc.vector.tensor_copy(goldT[:], goldT_ps[:])
            sdot = psr.tile([1, 32], F32, tag="cc")
            nc.tensor.matmul(sdot[:], lhsT=esp_s[:], rhs=state[:],
                             start=True, stop=True)
            lnf = mp.tile([1, 32], F32)
            nc.scalar.activation(lnf[:], sdot[:],
                                 mybir.ActivationFunctionType.Ln)
            if ridx > 0:
                lnring = mp.tile([1, 32 * nr], F32)
                nc.scalar.activation(lnring[:], ring[:, :32 * nr],
                                     mybir.ActivationFunctionType.Ln)
                lnsum = mp.tile([1, 32], F32)
                nc.vector.tensor_reduce(
                    lnsum[:],
                    lnring[:].rearrange("p (r b) -> p b r", b=32),
                    axis=mybir.AxisListType.X, op=mybir.AluOpType.add)
                y0 = mp.tile([1, 32], F32)
                # lnsum holds sum of ln(1/c): logZ = lnf - lnsum + (s-1)*kappa
                nc.vector.tensor_tensor(out=y0[:], in0=lnf[:], in1=lnsum[:],
                                        op=mybir.AluOpType.subtract)
            else:
                y0 = lnf
            y1 = mp.tile([1, 32], F32)
            nc.vector.tensor_tensor(out=y1[:], in0=y0[:], in1=goldT[:],
                                    op=mybir.AluOpType.subtract)
            nc.sync.dma_start(out[:], y1[:])
            rep_cm.__exit__(None, None, None)
    nc.compile()
    return nc


_NC_CACHE = {}


def _get_nc(s_len):
    if s_len not in _NC_CACHE:
        _NC_CACHE[s_len] = _build_nc(s_len)
    return _NC_CACHE[s_len]


def _host_constants(cdt, types0, types1, start_t, stop_t):
    trans = np.asarray(cdt, np.float64)[np.asarray(types0), np.asarray(types1)]
    kappa = float(np.log(np.exp(trans).sum(1)).mean() + 0.5)
    mt_np = np.exp(trans - kappa).astype(np.float32)          # [T,T] lhsT=[i,j]
    est_np = np.exp(np.asarray(start_t, np.float32)).reshape(T, 1)
    esp_np = np.exp(np.asarray(stop_t, np.float32)).reshape(T, 1)
    return mt_np, est_np, esp_np, kappa, trans


def kernel(feats, mask, tags, cdt_transitions, start_transitions,
           stop_transitions, types0, types1, s_len=None):
    feats = np.asarray(feats, np.float32)
    tags = np.asarray(tags, np.int64)
    s_len = feats.shape[1] if s_len is None else s_len
    mt_np, est_np, esp_np, kappa, trans = _host_constants(
        cdt_transitions, types0, types1, start_transitions, stop_transitions)
    start64 = np.asarray(start_transitions, np.float64)
    stop64 = np.asarray(stop_transitions, np.float64)
    # tags-only part of the gold score (host; touches no large tensors)
    gs = (trans[tags[:, :s_len - 1], tags[:, 1:s_len]].sum(1)
          + start64[tags[:, 0]] + stop64[tags[:, s_len - 1]])
    nc = _get_nc(s_len)
    iota_np = np.broadcast_to(np.arange(T, dtype=np.float32), (BC, T)).copy()
    in_maps = []
    for c in range(NCORES):
        in_maps.append({
            "fflat": np.ascontiguousarray(
                feats[c * BC:(c + 1) * BC, :s_len]).reshape(-1),
            "mt": mt_np, "ident": np.eye(32, dtype=np.float32),
            "est": est_np, "esp": esp_np,
            "onescol": np.ones((T, 1), np.float32),
            "onesrow": np.ones((1, T), np.float32),
            "iota": iota_np,
            "tagsf": tags[c * BC:(c + 1) * BC, :s_len].astype(np.float32),
        })
    res = run_bass_kernel_spmd(nc, in_maps, core_ids=list(range(NCORES)))
    outs = [res.results[c]["out"].reshape(BC) for c in range(NCORES)]
    nll = np.concatenate(outs).astype(np.float64)
    # logZ kappa correction and tags-only gold part
    nll = nll + (s_len - 1) * kappa - gs
    return nll.astype(np.float32)


# revision 10
# speedup vs baseline: 621.0011x; 2.1389x over previous
"""Trainium2 Bass kernel for the batched linear-chain CRF NLL.

Math: the reference computes, per batch row b,
    NLL[b] = logZ[b] - gold[b]
where logZ is the CRF forward (log-partition) over S=1024 steps with T=73
tags, and gold is the score of the labeled path.

Device strategy (pure data parallelism, batch 256 -> 32 rows x 8 cores):
  * Forward recurrence runs in exp-space:  A_{t+1} = E_t (.) (Mt^T A_t)
    with A in transposed layout [73 tags (partitions) x 32 batch (free)],
    Mt = exp(trans - kappa) stationary-ish on the TensorEngine, E_t =
    exp(feats_t)^T produced by PE transposes (matmul with identity) + one
    bulk ScalarEngine exp per 16 steps.  One VectorEngine tensor_tensor
    multiply per step carries the serial chain.
  * Numerical scaling: every RS steps a column-sum (ones matmul) +
    reciprocal + rank-1 broadcast matmul prescales a *future* E slice, so
    the rescale never adds latency to the serial chain.  log(c) factors
    are accumulated at the end from the stored reciprocals.
  * Gold score: two indirect-DMA gathers (feats at gold tags; a combined
    transition/start/stop table at host-computed flat indices), free-dim
    reduces, and a selection matmul to fold partial sums per row.

The kernel is self-contained: shapes/sharding hardcoded for
feats[256,1024,73], mask all-ones, tags[256,1024].
"""
import numpy as np

import concourse.mybir as mybir
import concourse.tile as tile
from concourse import bacc
import concourse.bass as bass
from concourse.bass_utils import run_bass_kernel_spmd

F32 = mybir.dt.float32
BF16 = mybir.dt.bfloat16
I32 = mybir.dt.int32

B, S, T = 256, 1024, 73
NCORES = 8
BC = B // NCORES          # 32 batch rows per core
CH = 64                   # feats steps per DMA chunk
TCH = 16                  # steps per PSUM transpose bank / ACT exp
RS = 32                   # rescale period (steps)


def _build_nc(s_len: int, reps: int = 1):
    """Build the per-core Bass program for a sequence of s_len steps."""
    n_chunks = s_len // CH
    n_resc = (s_len - 3) // RS + 1 if s_len > 32 else 1
    # rescale at steps t = RS-3, 2*RS-3, ... (t+2 stays inside t's chunk)
    resc_steps = set(range(RS - 3, s_len - 2, RS))
    nr = len(resc_steps)
    gw = 257 if s_len == S else (s_len + 1 + 2 + 3) // 4  # gather width/row
    # pad so 4*gw >= s_len+2 entries (trans pairs + start + stop)
    while 4 * gw < (s_len - 1) + 2:
        gw += 1

    nc = bacc.Bacc(None, target_bir_lowering=False)
    with tile.TileContext(nc) as tc:
        with tc.tile_pool(name="dram", bufs=1, space="DRAM") as dram:
            fflat = dram.tile([BC * s_len * T], BF16, kind="ExternalInput",
                              name="fflat", uniquify=False)
            mt = dram.tile([T, T], BF16, kind="ExternalInput", name="mt",
                           uniquify=False)
            ident = dram.tile([32, 32], BF16, kind="ExternalInput", name="ident",
                              uniquify=False)
            identf = dram.tile([32, 32], F32, kind="ExternalInput",
                               name="identf", uniquify=False)
            est = dram.tile([T, 1], F32, kind="ExternalInput", name="est",
                            uniquify=False)
            esp = dram.tile([T, 1], BF16, kind="ExternalInput", name="esp",
                            uniquify=False)
            onescol = dram.tile([T, 1], BF16, kind="ExternalInput", name="onescol",
                                uniquify=False)
            onesrow = dram.tile([1, T], F32, kind="ExternalInput", name="onesrow",
                                uniquify=False)
            iota = dram.tile([BC, T], F32, kind="ExternalInput", name="iota",
                             uniquify=False)
            tagsf = dram.tile([BC, s_len], F32, kind="ExternalInput",
                              name="tagsf", uniquify=False)
            out = dram.tile([1, BC], F32, kind="ExternalOutput", name="out",
                            uniquify=False)

        with (
            tc.tile_pool(name="const", bufs=1) as cp,
            tc.tile_pool(name="fnat", bufs=2) as fp,
            tc.tile_pool(name="et", bufs=3) as ep,
            tc.tile_pool(name="state", bufs=3) as stp,
            tc.tile_pool(name="psl", bufs=2) as pslp,
            tc.tile_pool(name="misc", bufs=1) as mp,
            tc.tile_pool(name="ps_s", bufs=2, space="PSUM") as pss,
            tc.tile_pool(name="ps_f", bufs=2, space="PSUM") as psf,
            tc.tile_pool(name="ps_r", bufs=1, space="PSUM") as psr,
            tc.tile_pool(name="ps_g", bufs=1, space="PSUM") as psg,
        ):
            # ---- constants into SBUF ----
            mt_s = cp.tile([T, T], BF16)
            nc.sync.dma_start(mt_s[:], mt[:])
            id128 = cp.tile([128, 32], BF16)
            id_s = id128[96:128, :]
            nc.sync.dma_start(id_s, ident[:])
            idf128 = cp.tile([128, 32], F32)
            idf_s = idf128[96:128, :]
            nc.sync.dma_start(idf_s, identf[:])
            est_s = cp.tile([T, 1], F32)
            nc.sync.dma_start(est_s[:], est[:])
            esp_s = cp.tile([T, 1], BF16)
            nc.sync.dma_start(esp_s[:], esp[:])
            oc_s = cp.tile([T, 1], BF16)
            nc.sync.dma_start(oc_s[:], onescol[:])
            or_s = cp.tile([1, T], F32)
            nc.sync.dma_start(or_s[:], onesrow[:])
            iota128 = cp.tile([128, T], F32)
            iota_s = iota128[96:128, :]
            nc.sync.dma_start(iota_s, iota[:])
            tags128 = cp.tile([128, s_len], F32)
            tags_s = tags128[96:128, :]
            nc.sync.dma_start(tags_s, tagsf[:])
            ring = mp.tile([1, 32 * max(nr, 1)], F32)
            partials128 = mp.tile([128, s_len], F32)
            partials = partials128[96:128, :]
            scratch128 = mp.tile([128, T], F32)
            scratch = scratch128[96:128, :]


            # ---- forward recurrence ----
            import contextlib
            rep_cm = (tc.For_i(0, reps, 1) if reps > 1
                      else contextlib.nullcontext())
            state = None
            ridx = 0
            pending = {}  # step -> prescaled E tile
            rep_cm.__enter__()
            for c in range(n_chunks):
                fn128 = fp.tile([128, CH * T], BF16, tag="fnat")
                fn = fn128[96:128, :]
                # feats[b, c*CH:(c+1)*CH, :] contiguous per row
                nc.sync.dma_start(
                    fn, fflat[:].rearrange("(b r) -> b r", b=BC)
                    [:, c * CH * T:(c + 1) * CH * T])
                et = ep.tile([T, CH * 32], F32, tag="et")
                for g in range(CH // TCH):
                    ftp = psf.tile([T, TCH * 32], F32, tag="ftp")
                    for k in range(TCH):
                        kk = g * TCH + k
                        nc.tensor.matmul(
                            ftp[:, k * 32:(k + 1) * 32],
                            lhsT=fn[:, kk * T:(kk + 1) * T], rhs=id_s,
                            start=True, stop=True, tile_position=(96, 0))
                    nc.scalar.activation(
                        et[:, g * TCH * 32:(g + 1) * TCH * 32], ftp[:],
                        mybir.ActivationFunctionType.Exp)
                for k in range(CH):
                    t = c * CH + k
                    esl = pending.pop(t, None)
                    esl = esl if esl is not None else et[:, k * 32:(k + 1) * 32]
                    if t == 0:
                        state = stp.tile([T, 32], BF16, tag="state")
                        nc.vector.tensor_scalar(
                            out=state[:], in0=esl, scalar1=est_s[:, 0:1],
                            scalar2=None, op0=mybir.AluOpType.mult)
                    else:
                        sp = pss.tile([T, 32], F32, tag="sp")
                        nc.tensor.matmul(sp[:], lhsT=mt_s[:], rhs=state[:],
                                         start=True, stop=True)
                        nstate = stp.tile([T, 32], BF16, tag="state")
                        nc.vector.tensor_tensor(out=nstate[:], in0=sp[:],
                                                in1=esl,
                                                op=mybir.AluOpType.mult)
                        state = nstate
                    # gold feat-score contribution for step t (off-chain)
                    nc.vector.scalar_tensor_tensor(
                        out=scratch, in0=iota_s,
                        scalar=tags_s[:, t:t + 1],
                        in1=fn[:, k * T:(k + 1) * T],
                        op0=mybir.AluOpType.is_equal,
                        op1=mybir.AluOpType.mult,
                        accum_out=partials[:, t:t + 1])
                    if t == 0:
                        continue
                    if t in resc_steps:
                        # rescale side-chain: prescale E slice for step t+2
                        cc = psr.tile([T, 32], F32, tag="cc")
                        nc.tensor.matmul(cc[0:1, :], lhsT=oc_s[:], rhs=state[:],
                                         start=True, stop=True)
                        rsl = ring[:, ridx * 32:(ridx + 1) * 32]
                        nc.vector.reciprocal(rsl, cc[0:1, :])
                        cb = psr.tile([T, 32], F32, tag="cc")
                        nc.tensor.matmul(cb[:], lhsT=or_s[:], rhs=rsl,
                                         start=True, stop=True)
                        k2 = k + 2
                        psl = pslp.tile([T, 32], F32, tag="psl")
                        nc.vector.tensor_tensor(
                            out=psl[:], in0=et[:, k2 * 32:(k2 + 1) * 32],
                            in1=cb[:], op=mybir.AluOpType.mult)
                        pending[t + 2] = psl[:]
                        ridx += 1

            # ---- finale: gold feat-score fold + transpose ----
            gold128 = mp.tile([128, 1], F32)
            gold_sb = gold128[96:128, :]
            nc.vector.tensor_reduce(gold_sb, partials,
                                    axis=mybir.AxisListType.X,
                                    op=mybir.AluOpType.add)
            goldT_ps = psg.tile([1, 32], F32)
            nc.tensor.matmul(goldT_ps[:], lhsT=gold_sb, rhs=idf_s,
                             start=True, stop=True, tile_position=(96, 0))
            goldT = mp.tile([1, 32], F32)
            nc.vector.tensor_copy(goldT[:], goldT_ps[:])
            sdot = psr.tile([1, 32], F32, tag="cc")
            nc.tensor.matmul(sdot[:], lhsT=esp_s[:], rhs=state[:],
                             start=True, stop=True)
            lnf = mp.tile([1, 32], F32)
            nc.scalar.activation(lnf[:], sdot[:],
                                 mybir.ActivationFunctionType.Ln)
            if ridx > 0:
                lnring = mp.tile([1, 32 * nr], F32)
                nc.scalar.activation(lnring[:], ring[:, :32 * nr],
                                     mybir.ActivationFunctionType.Ln)
                lnsum = mp.tile([1, 32], F32)
                nc.vector.tensor_reduce(
                    lnsum[:],
                    lnring[:].rearrange("p (r b) -> p b r", b=32),
                    axis=mybir.AxisListType.X, op=mybir.AluOpType.add)
                y0 = mp.tile([1, 32], F32)
                # lnsum holds sum of ln(1/c): logZ = lnf - lnsum + (s-1)*kappa
                nc.vector.tensor_tensor(out=y0[:], in0=lnf[:], in1=lnsum[:],
                                        op=mybir.AluOpType.subtract)
            else:
                y0 = lnf
            y1 = mp.tile([1, 32], F32)
            nc.vector.tensor_tensor(out=y1[:], in0=y0[:], in1=goldT[:],
                                    op=mybir.AluOpType.subtract)
            nc.sync.dma_start(out[:], y1[:])
            rep_cm.__exit__(None, None, None)
    nc.compile()
    return nc


_NC_CACHE = {}


def _get_nc(s_len):
    if s_len not in _NC_CACHE:
        _NC_CACHE[s_len] = _build_nc(s_len)
    return _NC_CACHE[s_len]


def _host_constants(cdt, types0, types1, start_t, stop_t):
    import ml_dtypes
    trans = np.asarray(cdt, np.float64)[np.asarray(types0), np.asarray(types1)]
    kappa = float(np.log(np.exp(trans).sum(1)).mean() + 0.5)
    mt_np = np.exp(trans - kappa).astype(ml_dtypes.bfloat16)  # [T,T] lhsT=[i,j]
    est_np = np.exp(np.asarray(start_t, np.float32)).reshape(T, 1)
    esp_np = np.exp(np.asarray(stop_t, np.float32)).reshape(T, 1).astype(
        ml_dtypes.bfloat16)
    return mt_np, est_np, esp_np, kappa, trans


def kernel(feats, mask, tags, cdt_transitions, start_transitions,
           stop_transitions, types0, types1, s_len=None):
    feats = np.asarray(feats, np.float32)
    tags = np.asarray(tags, np.int64)
    s_len = feats.shape[1] if s_len is None else s_len
    mt_np, est_np, esp_np, kappa, trans = _host_constants(
        cdt_transitions, types0, types1, start_transitions, stop_transitions)
    start64 = np.asarray(start_transitions, np.float64)
    stop64 = np.asarray(stop_transitions, np.float64)
    # tags-only part of the gold score (host; touches no large tensors)
    gs = (trans[tags[:, :s_len - 1], tags[:, 1:s_len]].sum(1)
          + start64[tags[:, 0]] + stop64[tags[:, s_len - 1]])
    import ml_dtypes
    nc = _get_nc(s_len)
    iota_np = np.broadcast_to(np.arange(T, dtype=np.float32), (BC, T)).copy()
    feats16 = feats[:, :s_len].astype(ml_dtypes.bfloat16)
    in_maps = []
    for c in range(NCORES):
        in_maps.append({
            "fflat": np.ascontiguousarray(
                feats16[c * BC:(c + 1) * BC]).reshape(-1),
            "mt": mt_np, "ident": np.eye(32, dtype=ml_dtypes.bfloat16),
            "identf": np.eye(32, dtype=np.float32),
            "est": est_np, "esp": esp_np,
            "onescol": np.ones((T, 1), ml_dtypes.bfloat16),
            "onesrow": np.ones((1, T), np.float32),
            "iota": iota_np,
            "tagsf": tags[c * BC:(c + 1) * BC, :s_len].astype(np.float32),
        })
    res = run_bass_kernel_spmd(nc, in_maps, core_ids=list(range(NCORES)))
    outs = [res.results[c]["out"].reshape(BC) for c in range(NCORES)]
    nll = np.concatenate(outs).astype(np.float64)
    # logZ kappa correction and tags-only gold part
    nll = nll + (s_len - 1) * kappa - gs
    return nll.astype(np.float32)


# revision 11
# speedup vs baseline: 657.0654x; 1.0581x over previous
"""Trainium2 Bass kernel for the batched linear-chain CRF NLL.

Math: the reference computes, per batch row b,
    NLL[b] = logZ[b] - gold[b]
where logZ is the CRF forward (log-partition) over S=1024 steps with T=73
tags, and gold is the score of the labeled path.

Device strategy (pure data parallelism, batch 256 -> 32 rows x 8 cores):
  * Forward recurrence runs in exp-space:  A_{t+1} = E_t (.) (Mt^T A_t)
    with A in transposed layout [73 tags (partitions) x 32 batch (free)],
    Mt = exp(trans - kappa) stationary-ish on the TensorEngine, E_t =
    exp(feats_t)^T produced by PE transposes (matmul with identity) + one
    bulk ScalarEngine exp per 16 steps.  One VectorEngine tensor_tensor
    multiply per step carries the serial chain.
  * Numerical scaling: every RS steps a column-sum (ones matmul) +
    reciprocal + rank-1 broadcast matmul prescales a *future* E slice, so
    the rescale never adds latency to the serial chain.  log(c) factors
    are accumulated at the end from the stored reciprocals.
  * Gold score: two indirect-DMA gathers (feats at gold tags; a combined
    transition/start/stop table at host-computed flat indices), free-dim
    reduces, and a selection matmul to fold partial sums per row.

The kernel is self-contained: shapes/sharding hardcoded for
feats[256,1024,73], mask all-ones, tags[256,1024].
"""
import numpy as np

import concourse.mybir as mybir
import concourse.tile as tile
from concourse import bacc
import concourse.bass as bass
from concourse.bass_utils import run_bass_kernel_spmd

F32 = mybir.dt.float32
BF16 = mybir.dt.bfloat16
I32 = mybir.dt.int32

B, S, T = 256, 1024, 73
NCORES = 8
BC = B // NCORES          # 32 batch rows per core
CH = 64                   # feats steps per DMA chunk
TCH = 16                  # steps per PSUM transpose bank / ACT exp
RS = 32                   # rescale period (steps)


def _build_nc(s_len: int, reps: int = 1):
    """Build the per-core Bass program for a sequence of s_len steps."""
    n_chunks = s_len // CH
    n_resc = (s_len - 3) // RS + 1 if s_len > 32 else 1
    # rescale at steps t = RS-3, 2*RS-3, ... (t+2 stays inside t's chunk)
    resc_steps = set(range(RS - 3, s_len - 2, RS))
    nr = len(resc_steps)
    gw = 257 if s_len == S else (s_len + 1 + 2 + 3) // 4  # gather width/row
    # pad so 4*gw >= s_len+2 entries (trans pairs + start + stop)
    while 4 * gw < (s_len - 1) + 2:
        gw += 1

    nc = bacc.Bacc(None, target_bir_lowering=False)
    with tile.TileContext(nc) as tc:
        with tc.tile_pool(name="dram", bufs=1, space="DRAM") as dram:
            fflat = dram.tile([BC * s_len * T], BF16, kind="ExternalInput",
                              name="fflat", uniquify=False)
            mt = dram.tile([T, T], BF16, kind="ExternalInput", name="mt",
                           uniquify=False)
            ident = dram.tile([32, 32], BF16, kind="ExternalInput", name="ident",
                              uniquify=False)
            identf = dram.tile([32, 32], F32, kind="ExternalInput",
                               name="identf", uniquify=False)
            est = dram.tile([T, 1], F32, kind="ExternalInput", name="est",
                            uniquify=False)
            esp = dram.tile([T, 1], BF16, kind="ExternalInput", name="esp",
                            uniquify=False)
            onescol = dram.tile([T, 1], BF16, kind="ExternalInput", name="onescol",
                                uniquify=False)
            onesrow = dram.tile([1, T], F32, kind="ExternalInput", name="onesrow",
                                uniquify=False)
            iota = dram.tile([BC, T], F32, kind="ExternalInput", name="iota",
                             uniquify=False)
            tagsf = dram.tile([BC, s_len], F32, kind="ExternalInput",
                              name="tagsf", uniquify=False)
            out = dram.tile([1, BC], F32, kind="ExternalOutput", name="out",
                            uniquify=False)

        with (
            tc.tile_pool(name="const", bufs=1) as cp,
            tc.tile_pool(name="fnat", bufs=2) as fp,
            tc.tile_pool(name="et", bufs=3) as ep,
            tc.tile_pool(name="state", bufs=3) as stp,
            tc.tile_pool(name="psl", bufs=2) as pslp,
            tc.tile_pool(name="misc", bufs=1) as mp,
            tc.tile_pool(name="ps_s", bufs=2, space="PSUM") as pss,
            tc.tile_pool(name="ps_f", bufs=2, space="PSUM") as psf,
            tc.tile_pool(name="ps_r", bufs=1, space="PSUM") as psr,
            tc.tile_pool(name="ps_g", bufs=1, space="PSUM") as psg,
        ):
            # ---- constants into SBUF ----
            mt_s = cp.tile([T, T], BF16)
            nc.sync.dma_start(mt_s[:], mt[:])
            id128 = cp.tile([128, 32], BF16)
            id_s = id128[96:128, :]
            nc.sync.dma_start(id_s, ident[:])
            idf128 = cp.tile([128, 32], F32)
            idf_s = idf128[96:128, :]
            nc.sync.dma_start(idf_s, identf[:])
            est_s = cp.tile([T, 1], F32)
            nc.sync.dma_start(est_s[:], est[:])
            esp_s = cp.tile([T, 1], BF16)
            nc.sync.dma_start(esp_s[:], esp[:])
            oc_s = cp.tile([T, 1], BF16)
            nc.sync.dma_start(oc_s[:], onescol[:])
            or_s = cp.tile([1, T], F32)
            nc.sync.dma_start(or_s[:], onesrow[:])
            iota128 = cp.tile([128, T], F32)
            iota_s = iota128[96:128, :]
            nc.sync.dma_start(iota_s, iota[:])
            tags128 = cp.tile([128, s_len], F32)
            tags_s = tags128[96:128, :]
            nc.sync.dma_start(tags_s, tagsf[:])
            ring = mp.tile([1, 32 * max(nr, 1)], F32)
            partials128 = mp.tile([128, s_len], F32)
            partials = partials128[96:128, :]
            scratch128 = mp.tile([128, T], F32)
            scratch = scratch128[96:128, :]


            # ---- forward recurrence ----
            import contextlib
            rep_cm = (tc.For_i(0, reps, 1) if reps > 1
                      else contextlib.nullcontext())
            state = None
            ridx = 0
            pending = {}  # step -> prescaled E tile
            rep_cm.__enter__()
            for c in range(n_chunks):
                fn128 = fp.tile([128, CH * T], BF16, tag="fnat")
                fn = fn128[96:128, :]
                # feats[b, c*CH:(c+1)*CH, :] contiguous per row
                nc.sync.dma_start(
                    fn, fflat[:].rearrange("(b r) -> b r", b=BC)
                    [:, c * CH * T:(c + 1) * CH * T])
                et = ep.tile([T, CH * 32], F32, tag="et")
                for g in range(CH // TCH):
                    ftp = psf.tile([T, TCH * 32], F32, tag="ftp")
                    for k in range(TCH):
                        kk = g * TCH + k
                        nc.tensor.matmul(
                            ftp[:, k * 32:(k + 1) * 32],
                            lhsT=fn[:, kk * T:(kk + 1) * T], rhs=id_s,
                            start=True, stop=True, tile_position=(96, 0))
                    nc.scalar.activation(
                        et[:, g * TCH * 32:(g + 1) * TCH * 32], ftp[:],
                        mybir.ActivationFunctionType.Exp)
                for k in range(CH):
                    t = c * CH + k
                    esl = pending.pop(t, None)
                    esl = esl if esl is not None else et[:, k * 32:(k + 1) * 32]
                    if t == 0:
                        state = stp.tile([T, 32], BF16, tag="state")
                        nc.vector.tensor_scalar(
                            out=state[:], in0=esl, scalar1=est_s[:, 0:1],
                            scalar2=None, op0=mybir.AluOpType.mult)
                    else:
                        sp = pss.tile([T, 32], F32, tag="sp")
                        nc.tensor.matmul(sp[:], lhsT=mt_s[:], rhs=state[:],
                                         start=True, stop=True)
                        nstate = stp.tile([T, 32], BF16, tag="state")
                        nc.vector.tensor_tensor(out=nstate[:], in0=sp[:],
                                                in1=esl,
                                                op=mybir.AluOpType.mult)
                        state = nstate
                    # gold feat-score contribution for step t (off-chain)
                    nc.vector.scalar_tensor_tensor(
                        out=scratch, in0=iota_s,
                        scalar=tags_s[:, t:t + 1],
                        in1=fn[:, k * T:(k + 1) * T],
                        op0=mybir.AluOpType.is_equal,
                        op1=mybir.AluOpType.mult,
                        accum_out=partials[:, t:t + 1])
                    if t == 0:
                        continue
                    if t in resc_steps:
                        # rescale side-chain: prescale E slice for step t+2
                        cc = psr.tile([T, 32], F32, tag="cc")
                        nc.tensor.matmul(cc[0:1, :], lhsT=oc_s[:], rhs=state[:],
                                         start=True, stop=True)
                        rsl = ring[:, ridx * 32:(ridx + 1) * 32]
                        nc.vector.reciprocal(rsl, cc[0:1, :])
                        cb = psr.tile([T, 32], F32, tag="cc")
                        nc.tensor.matmul(cb[:], lhsT=or_s[:], rhs=rsl,
                                         start=True, stop=True)
                        k2 = k + 2
                        psl = pslp.tile([T, 32], F32, tag="psl")
                        nc.vector.tensor_tensor(
                            out=psl[:], in0=et[:, k2 * 32:(k2 + 1) * 32],
                            in1=cb[:], op=mybir.AluOpType.mult)
                        pending[t + 2] = psl[:]
                        ridx += 1

            # ---- finale: gold feat-score fold + transpose ----
            gold128 = mp.tile([128, 1], F32)
            gold_sb = gold128[96:128, :]
            nc.vector.tensor_reduce(gold_sb, partials,
                                    axis=mybir.AxisListType.X,
                                    op=mybir.AluOpType.add)
            goldT_ps = psg.tile([1, 32], F32)
            nc.tensor.matmul(goldT_ps[:], lhsT=gold_sb, rhs=idf_s,
                             start=True, stop=True, tile_position=(96, 0))
            goldT = mp.tile([1, 32], F32)
            nc.vector.tensor_copy(goldT[:], goldT_ps[:])
            sdot = psr.tile([1, 32], F32, tag="cc")
            nc.tensor.matmul(sdot[:], lhsT=esp_s[:], rhs=state[:],
                             start=True, stop=True)
            lnf = mp.tile([1, 32], F32)
            nc.scalar.activation(lnf[:], sdot[:],
                                 mybir.ActivationFunctionType.Ln)
            if ridx > 0:
                lnring = mp.tile([1, 32 * nr], F32)
                nc.scalar.activation(lnring[:], ring[:, :32 * nr],
                                     mybir.ActivationFunctionType.Ln)
                lnsum = mp.tile([1, 32], F32)
                nc.vector.tensor_reduce(
                    lnsum[:],
                    lnring[:].rearrange("p (r b) -> p b r", b=32),
                    axis=mybir.AxisListType.X, op=mybir.AluOpType.add)
                y0 = mp.tile([1, 32], F32)
                # lnsum holds sum of ln(1/c): logZ = lnf - lnsum + (s-1)*kappa
                nc.vector.tensor_tensor(out=y0[:], in0=lnf[:], in1=lnsum[:],
                                        op=mybir.AluOpType.subtract)
            else:
                y0 = lnf
            y1 = mp.tile([1, 32], F32)
            nc.vector.tensor_tensor(out=y1[:], in0=y0[:], in1=goldT[:],
                                    op=mybir.AluOpType.subtract)
            nc.sync.dma_start(out[:], y1[:])
            rep_cm.__exit__(None, None, None)
    nc.compile()
    return nc


_NC_CACHE = {}


def _get_nc(s_len):
    if s_len not in _NC_CACHE:
        _NC_CACHE[s_len] = _build_nc(s_len)
    return _NC_CACHE[s_len]


def _host_constants(cdt, types0, types1, start_t, stop_t):
    import ml_dtypes
    trans = np.asarray(cdt, np.float64)[np.asarray(types0), np.asarray(types1)]
    kappa = float(np.log(np.exp(trans).sum(1)).mean() + 0.5)
    mtf = np.exp(trans - kappa)
    mt_np = mtf.astype(ml_dtypes.bfloat16)  # [T,T] lhsT=[i,j]
    # systematic log-bias of the bf16-rounded transition matrix: the same
    # matrix multiplies the state every step, so its rounding accumulates
    # linearly; cancel the row-mean log ratio on the host.
    delta = float(np.log(mt_np.astype(np.float64).sum(1) / mtf.sum(1)).mean())
    est_np = np.exp(np.asarray(start_t, np.float32)).reshape(T, 1)
    esp_np = np.exp(np.asarray(stop_t, np.float32)).reshape(T, 1).astype(
        ml_dtypes.bfloat16)
    return mt_np, est_np, esp_np, kappa - delta, trans


def kernel(feats, mask, tags, cdt_transitions, start_transitions,
           stop_transitions, types0, types1, s_len=None):
    feats = np.asarray(feats, np.float32)
    tags = np.asarray(tags, np.int64)
    s_len = feats.shape[1] if s_len is None else s_len
    mt_np, est_np, esp_np, kappa, trans = _host_constants(
        cdt_transitions, types0, types1, start_transitions, stop_transitions)
    start64 = np.asarray(start_transitions, np.float64)
    stop64 = np.asarray(stop_transitions, np.float64)
    # tags-only part of the gold score (host; touches no large tensors)
    gs = (trans[tags[:, :s_len - 1], tags[:, 1:s_len]].sum(1)
          + start64[tags[:, 0]] + stop64[tags[:, s_len - 1]])
    import ml_dtypes
    nc = _get_nc(s_len)
    iota_np = np.broadcast_to(np.arange(T, dtype=np.float32), (BC, T)).copy()
    feats16 = feats[:, :s_len].astype(ml_dtypes.bfloat16)
    in_maps = []
    for c in range(NCORES):
        in_maps.append({
            "fflat": np.ascontiguousarray(
                feats16[c * BC:(c + 1) * BC]).reshape(-1),
            "mt": mt_np, "ident": np.eye(32, dtype=ml_dtypes.bfloat16),
            "identf": np.eye(32, dtype=np.float32),
            "est": est_np, "esp": esp_np,
            "onescol": np.ones((T, 1), ml_dtypes.bfloat16),
            "onesrow": np.ones((1, T), np.float32),
            "iota": iota_np,
            "tagsf": tags[c * BC:(c + 1) * BC, :s_len].astype(np.float32),
        })
    res = run_bass_kernel_spmd(nc, in_maps, core_ids=list(range(NCORES)))
    outs = [res.results[c]["out"].reshape(BC) for c in range(NCORES)]
    nll = np.concatenate(outs).astype(np.float64)
    # logZ kappa correction and tags-only gold part
    nll = nll + (s_len - 1) * kappa - gs
    return nll.astype(np.float32)
